# revision 7
# baseline (speedup 1.0000x reference)
"""Trainium2 Bass kernel for the quirky-reshape MultiHeadSelfAttention layer.

Reference math (B=1, S=2048, H=768):
    qkv = x @ W_qkv                  # (S, 2304)
    col c' = h*36 + t ; h in [0,64) "heads", t in [0,36): q=t<12, k=12<=t<24, v=t>=24
    per head h (d_k=12):  A_h = softmax(Q_h K_h^T / sqrt(12));  O_h = A_h V_h
    values[:, h*12+d] = O_h[:, d];   out = values @ W_o

Sharding: 8 heads per core (tensor-parallel over the 64-head axis).
Each core computes a rank-96 partial of the output projection; the host
sums the 8 partials (the "all-reduce on output" done at unshard time).

Per-core device pipeline (all fp32):
  1. QKV projection with host-prepacked weights so Q^T/K^T land at the
     row-tile base partitions (32*s) needed for small-K (=12) matmul
     packing, and V lands as [j, 13] blocks with a built-in ones column
     (the ones column makes attention@V also produce the softmax
     denominator D).
  2. Per (i-chunk 512, j-block 128, head-group {3,3,2}): transposed
     logits A^T[j,i] via row-tiled K=12 matmuls into a 3-bank PSUM span,
     one Exp ACTIVATE over the whole span (scale=1/sqrt(12) folded in),
     then attention@V matmuls col-tiled 4-heads-per-PSUM-bank with
     interleaved per-element PSUM accumulation over j-blocks.
  3. Late softmax normalization: reciprocal of D rows, partition-
     broadcast of 1/D via a selector matmul, one elementwise multiply.
  4. Output projection with host-prepacked (zero-padded) W_o rows.
"""

import numpy as np

import concourse.bass as bass
import concourse.mybir as mybir
import concourse.tile as tile
from concourse import bacc
from concourse.bass_utils import run_bass_kernel_spmd

F32 = mybir.dt.float32

S = 2048
H = 768
DK = 12            # per-head dim (reference N_HEADS)
N_HEADS = 64       # effective heads (reference head_dim axis)
HEADS_PER_CORE = 8
N_CORES = 8
SCALE = 1.0 / float(np.sqrt(DK))
# logits head groups: (group, n_slots); slots at base partitions 0/32/64
GROUPS = ((0, 3), (1, 3), (2, 2))


def _head_of(g, s):
    return 3 * g + s if g < 2 else 6 + s


def build_program():
    nc = bacc.Bacc("TRN2", target_bir_lowering=False, debug=False)

    xt_d = nc.dram_tensor("xt", [H, S], F32, kind="ExternalInput")
    wqk_d = nc.dram_tensor("wqk", [H, 2, 3, 128], F32, kind="ExternalInput")
    wv_d = nc.dram_tensor("wv", [H, 96], F32, kind="ExternalInput")
    wo_d = nc.dram_tensor("wo", [2, 128, H], F32, kind="ExternalInput")
    sel_d = nc.dram_tensor("sel", [128, 128], F32, kind="ExternalInput")
    out_d = nc.dram_tensor("out", [S, H], F32, kind="ExternalOutput")

    with tile.TileContext(nc) as tc:
        with tc.tile_pool(name="const", bufs=1) as cpool:
            xt = cpool.tile([128, 6, S], F32, tag="xt")
            wqk = cpool.tile([128, 6, 2, 3, 128], F32, tag="wqk")
            wv = cpool.tile([128, 6, 96], F32, tag="wv")
            wo = cpool.tile([128, 2, H], F32, tag="wo")
            sel = cpool.tile([128, 128], F32, tag="sel")
            qkt = cpool.tile([128, 2, 3, S], F32, tag="qkt")
            vsb = cpool.tile([128, 16, 8, 13], F32, tag="vsb")
            vhat = cpool.tile([128, 2, S], F32, tag="vhat")

            xt_r = xt_d.rearrange("(hb p) s -> p hb s", p=128)
            for hb in range(6):
                nc.sync.dma_start(xt[:, hb, :], xt_r[:, hb, :])
            nc.sync.dma_start(wqk[:], wqk_d.rearrange("(hb p) t g m -> p hb t g m", p=128))
            nc.sync.dma_start(wv[:], wv_d.rearrange("(hb p) n -> p hb n", p=128))
            nc.sync.dma_start(wo[:], wo_d.rearrange("b p o -> p b o"))
            nc.sync.dma_start(sel[:], sel_d[:])
            nc.vector.memset(vhat[:], 0.0)
            # ones column (index 12) for the denominator trick; V columns
            # 0..11 get overwritten below.
            nc.vector.memset(vsb[:], 1.0)

            # ---- phase 1: QKV projection ----
            with tc.tile_pool(name="ps_qkv", bufs=2, space="PSUM") as ps_qkv:
                for t in range(2):          # 0 = Q^T, 1 = K^T
                    for g, nslots in GROUPS:
                        for ch in range(4):
                            p = ps_qkv.tile([128, 512], F32, tag="pqk")
                            for hb in range(6):
                                nc.tensor.matmul(
                                    p[:],
                                    lhsT=wqk[:, hb, t, g, :],
                                    rhs=xt[:, hb, ch * 512:(ch + 1) * 512],
                                    start=(hb == 0),
                                    stop=(hb == 5),
                                )
                            nc.vector.tensor_copy(
                                qkt[:, t, g, ch * 512:(ch + 1) * 512], p[:]
                            )
                for sb in range(16):
                    p = ps_qkv.tile([128, 512], F32, tag="pqk")
                    for hb in range(6):
                        nc.tensor.matmul(
                            p[:, :96],
                            lhsT=xt[:, hb, sb * 128:(sb + 1) * 128],
                            rhs=wv[:, hb, :],
                            start=(hb == 0),
                            stop=(hb == 5),
                        )
                    nc.vector.tensor_copy(
                        vsb[:, sb, :, 0:12],
                        p[:, :96].rearrange("p (h d) -> p h d", d=12),
                    )

            # ---- phase 2: attention ----
            with tc.tile_pool(name="ps_l", bufs=2, space="PSUM") as ps_l, \
                 tc.tile_pool(name="ps_av", bufs=1, space="PSUM") as ps_av, \
                 tc.tile_pool(name="esb", bufs=3) as esb:
                for ic in range(4):
                    av = [ps_av.tile([128, 512], F32, tag=f"av{b}", name=f"av{b}_{ic}")
                          for b in range(2)]
                    for jb in range(16):
                        for g, nslots in GROUPS:
                            L = ps_l.tile([128, 3, 512], F32, tag="L")
                            for s in range(nslots):
                                nc.tensor.matmul(
                                    L[:, s, :],
                                    lhsT=qkt[32 * s:32 * s + 12, 1, g,
                                             jb * 128:(jb + 1) * 128],
                                    rhs=qkt[32 * s:32 * s + 12, 0, g,
                                            ic * 512:(ic + 1) * 512],
                                    start=True,
                                    stop=True,
                                    tile_position=(32 * s, 0),
                                )
                            E = esb.tile([128, 3, 512], F32, tag="E")
                            nc.scalar.activation(
                                E[:, :nslots, :],
                                L[:, :nslots, :],
                                mybir.ActivationFunctionType.Exp,
                                scale=SCALE,
                            )
                            for s in range(nslots):
                                h = _head_of(g, s)
                                b, c = divmod(h, 4)
                                # has_written tracking is per-partition, so the
                                # four col-slots of one bank are independent
                                # accumulation groups (disjoint partitions).
                                nc.tensor.matmul(
                                    av[b][32 * c:32 * c + 13, :],
                                    lhsT=vsb[:, jb, h, :],
                                    rhs=E[:, s, :],
                                    start=(jb == 0),
                                    stop=(jb == 15),
                                    tile_position=(0, 32 * c),
                                    # CoreSim's group checker is not partition-
                                    # aware; the pending-zero numerics are.
                                    skip_group_check=True,
                                )
                    for b in range(2):
                        for c in range(4):
                            nc.vector.tensor_copy(
                                vhat[32 * c:32 * c + 13, b, ic * 512:(ic + 1) * 512],
                                av[b][32 * c:32 * c + 13, :],
                            )

            # ---- phase 3: normalize + output projection ----
            # sel broadcasts each head-quad's D row to all 32 rows of its
            # group (positive everywhere), so a full-tile reciprocal is safe.
            with tc.tile_pool(name="ps_fin", bufs=2, space="PSUM") as ps_fin, \
                 tc.tile_pool(name="osb", bufs=2) as opool:
                for b in range(2):
                    for ch in range(4):
                        bc = ps_fin.tile([128, 512], F32, tag="bc")
                        nc.tensor.matmul(
                            bc[:],
                            lhsT=sel[:],
                            rhs=vhat[:, b, ch * 512:(ch + 1) * 512],
                            start=True,
                            stop=True,
                        )
                        nc.vector.reciprocal(bc[:], bc[:])
                        nc.vector.tensor_tensor(
                            vhat[:, b, ch * 512:(ch + 1) * 512],
                            vhat[:, b, ch * 512:(ch + 1) * 512],
                            bc[:],
                            mybir.AluOpType.mult,
                        )
                for ib in range(16):
                    po = ps_fin.tile([128, 768], F32, tag="po")
                    for (o0, o1) in ((0, 512), (512, 768)):
                        for b in range(2):
                            nc.tensor.matmul(
                                po[:, o0:o1],
                                lhsT=vhat[:, b, ib * 128:(ib + 1) * 128],
                                rhs=wo[:, b, o0:o1],
                                start=(b == 0),
                                stop=(b == 1),
                            )
                    osb = opool.tile([128, 768], F32, tag="osb")
                    nc.vector.tensor_copy(osb[:], po[:])
                    nc.sync.dma_start(out_d[ib * 128:(ib + 1) * 128, :], osb[:])

    nc.compile()
    return nc


def make_core_inputs(x, W_qkv, W_o):
    """Host-side shard/prepack. Returns list of per-core input dicts."""
    x = np.asarray(x, np.float32)
    W_qkv = np.asarray(W_qkv, np.float32)
    W_o = np.asarray(W_o, np.float32)
    xt = np.ascontiguousarray(x.reshape(S, H).T)  # [H, S]

    sel = np.zeros((128, 128), np.float32)
    for s4 in range(4):
        sel[32 * s4 + 12, 32 * s4:32 * (s4 + 1)] = 1.0

    in_maps = []
    for core in range(N_CORES):
        wqk = np.zeros((H, 2, 3, 128), np.float32)
        wv = np.zeros((H, 96), np.float32)
        wo = np.zeros((2, 128, H), np.float32)
        for g, nslots in GROUPS:
            for s in range(nslots):
                h = _head_of(g, s)
                Hg = HEADS_PER_CORE * core + h
                for t in range(2):
                    wqk[:, t, g, 32 * s:32 * s + 12] = \
                        W_qkv[:, Hg * 36 + t * 12:Hg * 36 + (t + 1) * 12]
        for h in range(HEADS_PER_CORE):
            Hg = HEADS_PER_CORE * core + h
            wv[:, 12 * h:12 * (h + 1)] = W_qkv[:, Hg * 36 + 24:Hg * 36 + 36]
            b, c = divmod(h, 4)
            wo[b, 32 * c:32 * c + 12, :] = W_o[Hg * DK:(Hg + 1) * DK, :]
        in_maps.append({"xt": xt, "wqk": wqk, "wv": wv, "wo": wo, "sel": sel})
    return in_maps


_NC_CACHE = None


def kernel(x, W_qkv, W_o):
    global _NC_CACHE
    if _NC_CACHE is None:
        _NC_CACHE = build_program()
    nc = _NC_CACHE
    in_maps = make_core_inputs(x, W_qkv, W_o)
    res = run_bass_kernel_spmd(nc, in_maps, core_ids=list(range(N_CORES)))
    out = np.zeros((S, H), np.float64)
    for r in res.results:
        out += r["out"].astype(np.float64)
    return out.astype(np.float32).reshape(1, S, H)


# revision 10
# speedup vs baseline: 2.1434x; 2.1434x over previous
"""Trainium2 Bass kernel for the quirky-reshape MultiHeadSelfAttention layer.

Reference math (B=1, S=2048, H=768):
    qkv = x @ W_qkv                  # (S, 2304)
    col c' = h*36 + t ; h in [0,64) "heads", t in [0,36): q=t<12, k=12<=t<24, v=t>=24
    per head h (d_k=12):  A_h = softmax(Q_h K_h^T / sqrt(12));  O_h = A_h V_h
    values[:, h*12+d] = O_h[:, d];   out = values @ W_o

Sharding: 8 heads per core (tensor-parallel over the 64-head axis).
Each core computes a rank-96 partial of the output projection; the host
sums the 8 partials (the "all-reduce on output" done at unshard time).

Per-core device pipeline (all fp32):
  1. QKV projection with host-prepacked weights so Q^T/K^T land at the
     row-tile base partitions (32*s) needed for small-K (=12) matmul
     packing, and V lands as [j, 13] blocks with a built-in ones column
     (the ones column makes attention@V also produce the softmax
     denominator D).
  2. Per (i-chunk 512, j-block 128, head-group {3,3,2}): transposed
     logits A^T[j,i] via row-tiled K=12 matmuls into a 3-bank PSUM span,
     one Exp ACTIVATE over the whole span (scale=1/sqrt(12) folded in),
     then attention@V matmuls col-tiled 4-heads-per-PSUM-bank with
     interleaved per-element PSUM accumulation over j-blocks.
  3. Late softmax normalization: reciprocal of D rows, partition-
     broadcast of 1/D via a selector matmul, one elementwise multiply.
  4. Output projection with host-prepacked (zero-padded) W_o rows.
"""

import numpy as np

import concourse.bass as bass
import concourse.mybir as mybir
import concourse.tile as tile
from concourse import bacc
from concourse.bass_utils import run_bass_kernel_spmd

F32 = mybir.dt.float32
F32R = mybir.dt.float32r
BF16 = mybir.dt.bfloat16
FP16 = mybir.dt.float16

S = 2048
H = 768
DK = 12            # per-head dim (reference N_HEADS)
N_HEADS = 64       # effective heads (reference head_dim axis)
HEADS_PER_CORE = 8
N_CORES = 8
SCALE = 1.0 / float(np.sqrt(DK))
# logits head groups: (group, n_slots); slots at base partitions 0/32/64
GROUPS = ((0, 3), (1, 3), (2, 2))


def _head_of(g, s):
    return 3 * g + s if g < 2 else 6 + s


def build_program():
    nc = bacc.Bacc("TRN2", target_bir_lowering=False, debug=False)

    xt_d = nc.dram_tensor("xt", [H, S], F32R, kind="ExternalInput")
    wqk_d = nc.dram_tensor("wqk", [H, 2, 3, 128], F32R, kind="ExternalInput")
    wv_d = nc.dram_tensor("wv", [H, 96], F32R, kind="ExternalInput")
    wo_d = nc.dram_tensor("wo", [2, 128, H], F32R, kind="ExternalInput")
    sel_d = nc.dram_tensor("sel", [128, 128], F32R, kind="ExternalInput")
    out_d = nc.dram_tensor("out", [S, H], F32, kind="ExternalOutput")

    with tile.TileContext(nc) as tc:
        with tc.tile_pool(name="const", bufs=1) as cpool:
            xt = cpool.tile([128, 6, S], F32R, tag="xt")
            wqk = cpool.tile([128, 6, 2, 3, 128], F32R, tag="wqk")
            wv = cpool.tile([128, 6, 96], F32R, tag="wv")
            wo = cpool.tile([128, 2, H], F32R, tag="wo")
            sel = cpool.tile([128, 128], F32R, tag="sel")
            qkt = cpool.tile([128, 2, 3, S], F32R, tag="qkt")
            vsb = cpool.tile([128, 16, 8, 13], FP16, tag="vsb")
            vhat = cpool.tile([128, 2, S], F32R, tag="vhat")

            xt_r = xt_d.rearrange("(hb p) s -> p hb s", p=128)
            for hb in range(6):
                nc.sync.dma_start(xt[:, hb, :], xt_r[:, hb, :])
            nc.sync.dma_start(wqk[:], wqk_d.rearrange("(hb p) t g m -> p hb t g m", p=128))
            nc.sync.dma_start(wv[:], wv_d.rearrange("(hb p) n -> p hb n", p=128))
            nc.sync.dma_start(wo[:], wo_d.rearrange("b p o -> p b o"))
            nc.sync.dma_start(sel[:], sel_d[:])
            zscratch = cpool.tile([128, S], F32, tag="zscratch")
            nc.vector.memset(zscratch[:], 0.0)
            for b in range(2):
                nc.vector.tensor_copy(vhat[:, b, :], zscratch[:])
            # ones column (index 12) for the denominator trick; V columns
            # 0..11 get overwritten below.
            nc.vector.memset(vsb[:], 1.0)

            # ---- phase 1: QKV projection ----
            with tc.tile_pool(name="ps_qkv", bufs=2, space="PSUM") as ps_qkv:
                for t in range(2):          # 0 = Q^T, 1 = K^T
                    for g, nslots in GROUPS:
                        for ch in range(4):
                            p = ps_qkv.tile([128, 512], F32, tag="pqk")
                            for hb in range(6):
                                nc.tensor.matmul(
                                    p[:],
                                    lhsT=wqk[:, hb, t, g, :],
                                    rhs=xt[:, hb, ch * 512:(ch + 1) * 512],
                                    start=(hb == 0),
                                    stop=(hb == 5),
                                )
                            nc.vector.tensor_copy(
                                qkt[:, t, g, ch * 512:(ch + 1) * 512], p[:]
                            )
                for sb in range(16):
                    p = ps_qkv.tile([128, 512], F32, tag="pqk")
                    for hb in range(6):
                        nc.tensor.matmul(
                            p[:, :96],
                            lhsT=xt[:, hb, sb * 128:(sb + 1) * 128],
                            rhs=wv[:, hb, :],
                            start=(hb == 0),
                            stop=(hb == 5),
                        )
                    nc.vector.tensor_copy(
                        vsb[:, sb, :, 0:12],
                        p[:, :96].rearrange("p (h d) -> p h d", d=12),
                    )

            # ---- phase 2: attention ----
            with tc.tile_pool(name="ps_l", bufs=2, space="PSUM") as ps_l, \
                 tc.tile_pool(name="ps_av", bufs=1, space="PSUM") as ps_av, \
                 tc.tile_pool(name="esb", bufs=3) as esb:
                for ic in range(4):
                    av = [ps_av.tile([128, 512], F32, tag=f"av{b}", name=f"av{b}_{ic}")
                          for b in range(2)]
                    for jb in range(16):
                        for g, nslots in GROUPS:
                            L = ps_l.tile([128, 3, 512], F32, tag="L")
                            for s in range(nslots):
                                nc.tensor.matmul(
                                    L[:, s, :],
                                    lhsT=qkt[32 * s:32 * s + 12, 1, g,
                                             jb * 128:(jb + 1) * 128],
                                    rhs=qkt[32 * s:32 * s + 12, 0, g,
                                            ic * 512:(ic + 1) * 512],
                                    start=True,
                                    stop=True,
                                    tile_position=(32 * s, 0),
                                )
                            E = esb.tile([128, 3, 512], FP16, tag="E")
                            nc.scalar.activation(
                                E[:, :nslots, :],
                                L[:, :nslots, :],
                                mybir.ActivationFunctionType.Exp,
                                scale=SCALE,
                            )
                            for s in range(nslots):
                                h = _head_of(g, s)
                                b, c = divmod(h, 4)
                                # has_written tracking is per-partition, so the
                                # four col-slots of one bank are independent
                                # accumulation groups (disjoint partitions).
                                nc.tensor.matmul(
                                    av[b][32 * c:32 * c + 13, :],
                                    lhsT=vsb[:, jb, h, :],
                                    rhs=E[:, s, :],
                                    start=(jb == 0),
                                    stop=(jb == 15),
                                    tile_position=(0, 32 * c),
                                    # CoreSim's group checker is not partition-
                                    # aware; the pending-zero numerics are.
                                    skip_group_check=True,
                                )
                    for b in range(2):
                        for c in range(4):
                            nc.vector.tensor_copy(
                                vhat[32 * c:32 * c + 13, b, ic * 512:(ic + 1) * 512],
                                av[b][32 * c:32 * c + 13, :],
                            )

            # ---- phase 3: normalize + output projection ----
            # sel broadcasts each head-quad's D row to all 32 rows of its
            # group (positive everywhere), so a full-tile reciprocal is safe.
            with tc.tile_pool(name="ps_fin", bufs=2, space="PSUM") as ps_fin, \
                 tc.tile_pool(name="osb", bufs=2) as opool:
                for b in range(2):
                    for ch in range(4):
                        bc = ps_fin.tile([128, 512], F32, tag="bc")
                        nc.tensor.matmul(
                            bc[:],
                            lhsT=sel[:],
                            rhs=vhat[:, b, ch * 512:(ch + 1) * 512],
                            start=True,
                            stop=True,
                        )
                        nc.vector.reciprocal_approx_fast(bc[:], bc[:])
                        nc.vector.tensor_tensor(
                            vhat[:, b, ch * 512:(ch + 1) * 512],
                            vhat[:, b, ch * 512:(ch + 1) * 512],
                            bc[:],
                            mybir.AluOpType.mult,
                        )
                for ib in range(16):
                    po = ps_fin.tile([128, 768], F32, tag="po")
                    for (o0, o1) in ((0, 512), (512, 768)):
                        for b in range(2):
                            nc.tensor.matmul(
                                po[:, o0:o1],
                                lhsT=vhat[:, b, ib * 128:(ib + 1) * 128],
                                rhs=wo[:, b, o0:o1],
                                start=(b == 0),
                                stop=(b == 1),
                            )
                    osb = opool.tile([128, 768], F32, tag="osb")
                    nc.vector.tensor_copy(osb[:], po[:])
                    nc.sync.dma_start(out_d[ib * 128:(ib + 1) * 128, :], osb[:])

    nc.compile()
    return nc


def make_core_inputs(x, W_qkv, W_o):
    """Host-side shard/prepack. Returns list of per-core input dicts."""
    x = np.asarray(x, np.float32)
    W_qkv = np.asarray(W_qkv, np.float32)
    W_o = np.asarray(W_o, np.float32)
    xt = np.ascontiguousarray(x.reshape(S, H).T)  # [H, S]

    sel = np.zeros((128, 128), np.float32)
    for s4 in range(4):
        sel[32 * s4 + 12, 32 * s4:32 * (s4 + 1)] = 1.0

    in_maps = []
    for core in range(N_CORES):
        wqk = np.zeros((H, 2, 3, 128), np.float32)
        wv = np.zeros((H, 96), np.float32)
        wo = np.zeros((2, 128, H), np.float32)
        for g, nslots in GROUPS:
            for s in range(nslots):
                h = _head_of(g, s)
                Hg = HEADS_PER_CORE * core + h
                for t in range(2):
                    wqk[:, t, g, 32 * s:32 * s + 12] = \
                        W_qkv[:, Hg * 36 + t * 12:Hg * 36 + (t + 1) * 12]
        for h in range(HEADS_PER_CORE):
            Hg = HEADS_PER_CORE * core + h
            wv[:, 12 * h:12 * (h + 1)] = W_qkv[:, Hg * 36 + 24:Hg * 36 + 36]
            b, c = divmod(h, 4)
            wo[b, 32 * c:32 * c + 12, :] = W_o[Hg * DK:(Hg + 1) * DK, :]
        in_maps.append({"xt": xt, "wqk": wqk, "wv": wv, "wo": wo, "sel": sel})
    return in_maps


_NC_CACHE = None


def kernel(x, W_qkv, W_o):
    global _NC_CACHE
    if _NC_CACHE is None:
        _NC_CACHE = build_program()
    nc = _NC_CACHE
    in_maps = make_core_inputs(x, W_qkv, W_o)
    res = run_bass_kernel_spmd(nc, in_maps, core_ids=list(range(N_CORES)))
    out = np.zeros((S, H), np.float64)
    for r in res.results:
        out += r["out"].astype(np.float64)
    return out.astype(np.float32).reshape(1, S, H)


# revision 12
# speedup vs baseline: 2.4413x; 1.1390x over previous
"""Trainium2 Bass kernel for the quirky-reshape MultiHeadSelfAttention layer.

Reference math (B=1, S=2048, H=768):
    qkv = x @ W_qkv                  # (S, 2304)
    col c' = h*36 + t ; h in [0,64) "heads", t in [0,36): q=t<12, k=12<=t<24, v=t>=24
    per head h (d_k=12):  A_h = softmax(Q_h K_h^T / sqrt(12));  O_h = A_h V_h
    values[:, h*12+d] = O_h[:, d];   out = values @ W_o

Sharding: 8 heads per core (tensor-parallel over the 64-head axis).
Each core computes a rank-96 partial of the output projection; the host
sums the 8 partials (the "all-reduce on output" done at unshard time).

Per-core device pipeline (all fp32):
  1. QKV projection with host-prepacked weights so Q^T/K^T land at the
     row-tile base partitions (32*s) needed for small-K (=12) matmul
     packing, and V lands as [j, 13] blocks with a built-in ones column
     (the ones column makes attention@V also produce the softmax
     denominator D).
  2. Per (i-chunk 512, j-block 128, head-group {3,3,2}): transposed
     logits A^T[j,i] via row-tiled K=12 matmuls into a 3-bank PSUM span,
     one Exp ACTIVATE over the whole span (scale=1/sqrt(12) folded in),
     then attention@V matmuls col-tiled 4-heads-per-PSUM-bank with
     interleaved per-element PSUM accumulation over j-blocks.
  3. Late softmax normalization: reciprocal of D rows, partition-
     broadcast of 1/D via a selector matmul, one elementwise multiply.
  4. Output projection with host-prepacked (zero-padded) W_o rows.
"""

import numpy as np

import concourse.bass as bass
import concourse.mybir as mybir
import concourse.tile as tile
from concourse import bacc
from concourse.bass_utils import run_bass_kernel_spmd

F32 = mybir.dt.float32
F32R = mybir.dt.float32r
BF16 = mybir.dt.bfloat16
FP16 = mybir.dt.float16

S = 2048
H = 768
DK = 12            # per-head dim (reference N_HEADS)
N_HEADS = 64       # effective heads (reference head_dim axis)
HEADS_PER_CORE = 8
N_CORES = 8
SCALE = 1.0 / float(np.sqrt(DK))
# logits head groups: (group, n_slots); slots at base partitions 0/32/64
GROUPS = ((0, 3), (1, 3), (2, 2))


def _head_of(g, s):
    return 3 * g + s if g < 2 else 6 + s


def build_program():
    nc = bacc.Bacc("TRN2", target_bir_lowering=False, debug=False)

    xt_d = nc.dram_tensor("xt", [H, S], F32R, kind="ExternalInput")
    wqk_d = nc.dram_tensor("wqk", [H, 2, 3, 128], F32R, kind="ExternalInput")
    wv_d = nc.dram_tensor("wv", [H, 96], F32R, kind="ExternalInput")
    wo_d = nc.dram_tensor("wo", [2, 128, H], F32R, kind="ExternalInput")
    sel_d = nc.dram_tensor("sel", [128, 128], F32R, kind="ExternalInput")
    out_d = nc.dram_tensor("out", [S, H], F32, kind="ExternalOutput")

    with tile.TileContext(nc) as tc:
        with tc.tile_pool(name="const", bufs=1) as cpool:
            xt = cpool.tile([128, 6, S], F32R, tag="xt")
            wqk = cpool.tile([128, 6, 2, 3, 128], F32R, tag="wqk")
            wv = cpool.tile([128, 6, 96], F32R, tag="wv")
            wo = cpool.tile([128, 2, H], F32R, tag="wo")
            sel = cpool.tile([128, 128], F32R, tag="sel")
            qkt = cpool.tile([128, 2, 3, S], F32R, tag="qkt")
            vsb = cpool.tile([128, 16, 8, 13], FP16, tag="vsb")
            vhat = cpool.tile([128, 2, S], F32R, tag="vhat")

            xt_r = xt_d.rearrange("(hb p) s -> p hb s", p=128)
            for hb in range(6):
                nc.sync.dma_start(xt[:, hb, :], xt_r[:, hb, :])
            nc.sync.dma_start(wqk[:], wqk_d.rearrange("(hb p) t g m -> p hb t g m", p=128))
            nc.sync.dma_start(wv[:], wv_d.rearrange("(hb p) n -> p hb n", p=128))
            nc.sync.dma_start(wo[:], wo_d.rearrange("b p o -> p b o"))
            nc.sync.dma_start(sel[:], sel_d[:])
            zscratch = cpool.tile([128, S], F32, tag="zscratch")
            nc.vector.memset(zscratch[:], 0.0)
            for b in range(2):
                nc.vector.tensor_copy(vhat[:, b, :], zscratch[:])
            # ones column (index 12) for the denominator trick; V columns
            # 0..11 get overwritten below.
            nc.vector.memset(vsb[:], 1.0)

            # ---- phase 1: QKV projection ----
            with tc.tile_pool(name="ps_qkv", bufs=2, space="PSUM") as ps_qkv:
                for t in range(2):          # 0 = Q^T, 1 = K^T
                    for g, nslots in GROUPS:
                        for ch in range(4):
                            p = ps_qkv.tile([128, 512], F32, tag="pqk")
                            for hb in range(6):
                                nc.tensor.matmul(
                                    p[:],
                                    lhsT=wqk[:, hb, t, g, :],
                                    rhs=xt[:, hb, ch * 512:(ch + 1) * 512],
                                    start=(hb == 0),
                                    stop=(hb == 5),
                                )
                            nc.vector.tensor_copy(
                                qkt[:, t, g, ch * 512:(ch + 1) * 512], p[:]
                            )
                for sb in range(16):
                    p = ps_qkv.tile([128, 512], F32, tag="pqk")
                    for hb in range(6):
                        nc.tensor.matmul(
                            p[:, :96],
                            lhsT=xt[:, hb, sb * 128:(sb + 1) * 128],
                            rhs=wv[:, hb, :],
                            start=(hb == 0),
                            stop=(hb == 5),
                        )
                    nc.vector.tensor_copy(
                        vsb[:, sb, :, 0:12],
                        p[:, :96].rearrange("p (h d) -> p h d", d=12),
                    )

            # ---- phase 2: attention ----
            with tc.tile_pool(name="ps_l", bufs=2, space="PSUM") as ps_l, \
                 tc.tile_pool(name="ps_av", bufs=1, space="PSUM") as ps_av, \
                 tc.tile_pool(name="esb", bufs=4) as esb:
                def emit_av(av, E, g, nslots, jb):
                    for s in range(nslots):
                        h = _head_of(g, s)
                        b, c = divmod(h, 4)
                        # has_written tracking is per-partition, so the
                        # four col-slots of one bank are independent
                        # accumulation groups (disjoint partitions).
                        nc.tensor.matmul(
                            av[b][32 * c:32 * c + 13, :],
                            lhsT=vsb[:, jb, h, :],
                            rhs=E[:, s, :],
                            start=(jb == 0),
                            stop=(jb == 15),
                            tile_position=(0, 32 * c),
                            # CoreSim's group checker is not partition-
                            # aware; the pending-zero numerics are.
                            skip_group_check=True,
                        )

                for ic in range(4):
                    av = [ps_av.tile([128, 512], F32, tag=f"av{b}", name=f"av{b}_{ic}")
                          for b in range(2)]
                    # one-group software-pipeline skew: emit each group's
                    # attention@V after the NEXT group's logits, so the PE
                    # never stalls on the Exp it just fed.
                    pending = None
                    for jb in range(16):
                        for g, nslots in GROUPS:
                            L = ps_l.tile([128, 3, 512], F32, tag="L")
                            for s in range(nslots):
                                nc.tensor.matmul(
                                    L[:, s, :],
                                    lhsT=qkt[32 * s:32 * s + 12, 1, g,
                                             jb * 128:(jb + 1) * 128],
                                    rhs=qkt[32 * s:32 * s + 12, 0, g,
                                            ic * 512:(ic + 1) * 512],
                                    start=True,
                                    stop=True,
                                    tile_position=(32 * s, 0),
                                )
                            E = esb.tile([128, 3, 512], FP16, tag="E")
                            nc.scalar.activation(
                                E[:, :nslots, :],
                                L[:, :nslots, :],
                                mybir.ActivationFunctionType.Exp,
                                scale=SCALE,
                            )
                            if pending is not None:
                                emit_av(av, *pending)
                            pending = (E, g, nslots, jb)
                    emit_av(av, *pending)
                    for b in range(2):
                        for c in range(4):
                            nc.vector.tensor_copy(
                                vhat[32 * c:32 * c + 13, b, ic * 512:(ic + 1) * 512],
                                av[b][32 * c:32 * c + 13, :],
                            )

            # ---- phase 3: normalize + output projection ----
            # sel broadcasts each head-quad's D row to all 32 rows of its
            # group (positive everywhere), so a full-tile reciprocal is safe.
            with tc.tile_pool(name="ps_fin", bufs=2, space="PSUM") as ps_fin, \
                 tc.tile_pool(name="osb", bufs=2) as opool:
                for b in range(2):
                    for ch in range(4):
                        bc = ps_fin.tile([128, 512], F32, tag="bc")
                        nc.tensor.matmul(
                            bc[:],
                            lhsT=sel[:],
                            rhs=vhat[:, b, ch * 512:(ch + 1) * 512],
                            start=True,
                            stop=True,
                        )
                        nc.vector.reciprocal_approx_fast(bc[:], bc[:])
                        nc.vector.tensor_tensor(
                            vhat[:, b, ch * 512:(ch + 1) * 512],
                            vhat[:, b, ch * 512:(ch + 1) * 512],
                            bc[:],
                            mybir.AluOpType.mult,
                        )
                for ib in range(16):
                    po = ps_fin.tile([128, 768], F32, tag="po")
                    for (o0, o1) in ((0, 512), (512, 768)):
                        for b in range(2):
                            nc.tensor.matmul(
                                po[:, o0:o1],
                                lhsT=vhat[:, b, ib * 128:(ib + 1) * 128],
                                rhs=wo[:, b, o0:o1],
                                start=(b == 0),
                                stop=(b == 1),
                            )
                    osb = opool.tile([128, 768], F32, tag="osb")
                    nc.vector.tensor_copy(osb[:], po[:])
                    nc.sync.dma_start(out_d[ib * 128:(ib + 1) * 128, :], osb[:])

    nc.compile()
    return nc


def make_core_inputs(x, W_qkv, W_o):
    """Host-side shard/prepack. Returns list of per-core input dicts."""
    x = np.asarray(x, np.float32)
    W_qkv = np.asarray(W_qkv, np.float32)
    W_o = np.asarray(W_o, np.float32)
    xt = np.ascontiguousarray(x.reshape(S, H).T)  # [H, S]

    sel = np.zeros((128, 128), np.float32)
    for s4 in range(4):
        sel[32 * s4 + 12, 32 * s4:32 * (s4 + 1)] = 1.0

    in_maps = []
    for core in range(N_CORES):
        wqk = np.zeros((H, 2, 3, 128), np.float32)
        wv = np.zeros((H, 96), np.float32)
        wo = np.zeros((2, 128, H), np.float32)
        for g, nslots in GROUPS:
            for s in range(nslots):
                h = _head_of(g, s)
                Hg = HEADS_PER_CORE * core + h
                for t in range(2):
                    wqk[:, t, g, 32 * s:32 * s + 12] = \
                        W_qkv[:, Hg * 36 + t * 12:Hg * 36 + (t + 1) * 12]
        for h in range(HEADS_PER_CORE):
            Hg = HEADS_PER_CORE * core + h
            wv[:, 12 * h:12 * (h + 1)] = W_qkv[:, Hg * 36 + 24:Hg * 36 + 36]
            b, c = divmod(h, 4)
            wo[b, 32 * c:32 * c + 12, :] = W_o[Hg * DK:(Hg + 1) * DK, :]
        in_maps.append({"xt": xt, "wqk": wqk, "wv": wv, "wo": wo, "sel": sel})
    return in_maps


_NC_CACHE = None


def kernel(x, W_qkv, W_o):
    global _NC_CACHE
    if _NC_CACHE is None:
        _NC_CACHE = build_program()
    nc = _NC_CACHE
    in_maps = make_core_inputs(x, W_qkv, W_o)
    res = run_bass_kernel_spmd(nc, in_maps, core_ids=list(range(N_CORES)))
    out = np.zeros((S, H), np.float64)
    for r in res.results:
        out += r["out"].astype(np.float64)
    return out.astype(np.float32).reshape(1, S, H)


# revision 15
# speedup vs baseline: 2.5135x; 1.0296x over previous
"""Trainium2 Bass kernel for the quirky-reshape MultiHeadSelfAttention layer.

Reference math (B=1, S=2048, H=768):
    qkv = x @ W_qkv                  # (S, 2304)
    col c' = h*36 + t ; h in [0,64) "heads", t in [0,36): q=t<12, k=12<=t<24, v=t>=24
    per head h (d_k=12):  A_h = softmax(Q_h K_h^T / sqrt(12));  O_h = A_h V_h
    values[:, h*12+d] = O_h[:, d];   out = values @ W_o

Sharding: 8 heads per core (tensor-parallel over the 64-head axis).
Each core computes a rank-96 partial of the output projection; the host
sums the 8 partials (the "all-reduce on output" done at unshard time).

Per-core device pipeline (all fp32):
  1. QKV projection with host-prepacked weights so Q^T/K^T land at the
     row-tile base partitions (32*s) needed for small-K (=12) matmul
     packing, and V lands as [j, 13] blocks with a built-in ones column
     (the ones column makes attention@V also produce the softmax
     denominator D).
  2. Per (i-chunk 512, j-block 128, head-group {3,3,2}): transposed
     logits A^T[j,i] via row-tiled K=12 matmuls into a 3-bank PSUM span,
     one Exp ACTIVATE over the whole span (scale=1/sqrt(12) folded in),
     then attention@V matmuls col-tiled 4-heads-per-PSUM-bank with
     interleaved per-element PSUM accumulation over j-blocks.
  3. Late softmax normalization: reciprocal of D rows, partition-
     broadcast of 1/D via a selector matmul, one elementwise multiply.
  4. Output projection with host-prepacked (zero-padded) W_o rows.
"""

import numpy as np

import concourse.bass as bass
import concourse.mybir as mybir
import concourse.tile as tile
from concourse import bacc
from concourse.bass_utils import run_bass_kernel_spmd

F32 = mybir.dt.float32
F32R = mybir.dt.float32r
BF16 = mybir.dt.bfloat16
FP16 = mybir.dt.float16

S = 2048
H = 768
DK = 12            # per-head dim (reference N_HEADS)
N_HEADS = 64       # effective heads (reference head_dim axis)
HEADS_PER_CORE = 8
N_CORES = 8
SCALE = 1.0 / float(np.sqrt(DK))
# logits head groups: (group, n_slots); slots at base partitions 0/32/64
GROUPS = ((0, 3), (1, 3), (2, 2))


def _head_of(g, s):
    return 3 * g + s if g < 2 else 6 + s


def build_program():
    nc = bacc.Bacc("TRN2", target_bir_lowering=False, debug=False)

    xt_d = nc.dram_tensor("xt", [H, S], F32R, kind="ExternalInput")
    wqk_d = nc.dram_tensor("wqk", [H, 2, 3, 128], F32R, kind="ExternalInput")
    wv_d = nc.dram_tensor("wv", [H, 96], F32R, kind="ExternalInput")
    wo_d = nc.dram_tensor("wo", [2, 128, H], F32R, kind="ExternalInput")
    sel_d = nc.dram_tensor("sel", [128, 128], F32R, kind="ExternalInput")
    out_d = nc.dram_tensor("out", [S, H], F32, kind="ExternalOutput")

    with tile.TileContext(nc) as tc:
        with tc.tile_pool(name="const", bufs=1) as cpool:
            xt = cpool.tile([128, 6, S], F32R, tag="xt")
            wqk = cpool.tile([128, 6, 2, 3, 128], F32R, tag="wqk")
            wv = cpool.tile([128, 6, 96], F32R, tag="wv")
            wo = cpool.tile([128, 2, H], F32R, tag="wo")
            sel = cpool.tile([128, 128], F32R, tag="sel")
            qkt = cpool.tile([128, 2, 3, S], F32R, tag="qkt")
            vsb = cpool.tile([128, 16, 8, 13], FP16, tag="vsb")
            vhat = cpool.tile([128, 2, S], F32R, tag="vhat")

            # interleave weight/activation loads so the first QKV matmul can
            # start as soon as the first h-block of both is resident
            xt_r = xt_d.rearrange("(hb p) s -> p hb s", p=128)
            wqk_r = wqk_d.rearrange("(hb p) t g m -> p hb t g m", p=128)
            for hb in range(6):
                nc.sync.dma_start(wqk[:, hb], wqk_r[:, hb])
                nc.sync.dma_start(xt[:, hb, :], xt_r[:, hb, :])
            nc.sync.dma_start(wv[:], wv_d.rearrange("(hb p) n -> p hb n", p=128))
            nc.sync.dma_start(wo[:], wo_d.rearrange("b p o -> p b o"))
            nc.sync.dma_start(sel[:], sel_d[:])
            zscratch = cpool.tile([128, S], F32, tag="zscratch")
            nc.vector.memset(zscratch[:], 0.0)
            for b in range(2):
                nc.vector.tensor_copy(vhat[:, b, :], zscratch[:])
            # ones column (index 12) for the denominator trick; V columns
            # 0..11 get overwritten below.
            nc.vector.memset(vsb[:], 1.0)

            # ---- phase 1: QKV projection ----
            with tc.tile_pool(name="ps_qkv", bufs=2, space="PSUM") as ps_qkv:
                for t in range(2):          # 0 = Q^T, 1 = K^T
                    for g, nslots in GROUPS:
                        for ch in range(4):
                            p = ps_qkv.tile([128, 512], F32, tag="pqk")
                            for hb in range(6):
                                nc.tensor.matmul(
                                    p[:],
                                    lhsT=wqk[:, hb, t, g, :],
                                    rhs=xt[:, hb, ch * 512:(ch + 1) * 512],
                                    start=(hb == 0),
                                    stop=(hb == 5),
                                )
                            nc.vector.tensor_copy(
                                qkt[:, t, g, ch * 512:(ch + 1) * 512], p[:]
                            )
                for sb in range(16):
                    p = ps_qkv.tile([128, 512], F32, tag="pqk")
                    for hb in range(6):
                        nc.tensor.matmul(
                            p[:, :96],
                            lhsT=xt[:, hb, sb * 128:(sb + 1) * 128],
                            rhs=wv[:, hb, :],
                            start=(hb == 0),
                            stop=(hb == 5),
                        )
                    nc.vector.tensor_copy(
                        vsb[:, sb, :, 0:12],
                        p[:, :96].rearrange("p (h d) -> p h d", d=12),
                    )

            # ---- phase 2: attention, with per-chunk epilogue overlapped ----
            with tc.tile_pool(name="ps_l", bufs=2, space="PSUM") as ps_l, \
                 tc.tile_pool(name="ps_av", bufs=1, space="PSUM") as ps_av, \
                 tc.tile_pool(name="esb", bufs=6) as esb, \
                 tc.tile_pool(name="osb", bufs=2) as opool:
                def emit_av(av, E, g, nslots, jb):
                    for s in range(nslots):
                        h = _head_of(g, s)
                        b, c = divmod(h, 4)
                        # has_written tracking is per-partition, so the
                        # four col-slots of one bank are independent
                        # accumulation groups (disjoint partitions).
                        nc.tensor.matmul(
                            av[b][32 * c:32 * c + 13, :],
                            lhsT=vsb[:, jb, h, :],
                            rhs=E[:, s, :],
                            start=(jb == 0),
                            stop=(jb == 15),
                            tile_position=(0, 32 * c),
                            # CoreSim's group checker is not partition-
                            # aware; the pending-zero numerics are.
                            skip_group_check=True,
                        )

                def emit_epilogue(ic):
                    # softmax normalization + output projection + store for
                    # chunk ic. sel broadcasts each head-quad's D row to all
                    # 32 rows of its group (positive everywhere) so the
                    # full-tile fast reciprocal is safe. Uses the av-pool
                    # slots (free between accumulation rounds).
                    lo, hi = ic * 512, (ic + 1) * 512
                    for b in range(2):
                        bc = ps_av.tile([128, 512], F32, tag=f"av{b}",
                                        name=f"bc{b}_{ic}")
                        nc.tensor.matmul(bc[:], lhsT=sel[:], rhs=vhat[:, b, lo:hi],
                                         start=True, stop=True)
                        nc.vector.reciprocal_approx_fast(bc[:], bc[:])
                        nc.vector.tensor_tensor(vhat[:, b, lo:hi],
                                                vhat[:, b, lo:hi],
                                                bc[:], mybir.AluOpType.mult)
                    for ib4 in range(4):
                        ib = ic * 4 + ib4
                        poa = ps_av.tile([128, 512], F32, tag="av0", name=f"poa_{ib}")
                        pob = ps_av.tile([128, 512], F32, tag="av1", name=f"pob_{ib}")
                        for b in range(2):
                            nc.tensor.matmul(
                                poa[:],
                                lhsT=vhat[:, b, ib * 128:(ib + 1) * 128],
                                rhs=wo[:, b, 0:512],
                                start=(b == 0), stop=(b == 1),
                            )
                        for b in range(2):
                            nc.tensor.matmul(
                                pob[:, :256],
                                lhsT=vhat[:, b, ib * 128:(ib + 1) * 128],
                                rhs=wo[:, b, 512:768],
                                start=(b == 0), stop=(b == 1),
                            )
                        osb = opool.tile([128, 768], F32, tag="osb", name=f"osb_{ib}")
                        nc.vector.tensor_copy(osb[:, 0:512], poa[:])
                        nc.vector.tensor_copy(osb[:, 512:768], pob[:, :256])
                        nc.sync.dma_start(out_d[ib * 128:(ib + 1) * 128, :], osb[:])

                prev_ic = None  # chunk whose epilogue is still pending
                for ic in range(4):
                    av = None
                    # one-group software-pipeline skew: emit each group's
                    # attention@V after the NEXT group's logits, so the PE
                    # never stalls on the Exp it just fed.
                    pending = None
                    for jb in range(16):
                        for g, nslots in GROUPS:
                            L = ps_l.tile([128, 3, 512], F32, tag="L")
                            for s in range(nslots):
                                nc.tensor.matmul(
                                    L[:, s, :],
                                    lhsT=qkt[32 * s:32 * s + 12, 1, g,
                                             jb * 128:(jb + 1) * 128],
                                    rhs=qkt[32 * s:32 * s + 12, 0, g,
                                            ic * 512:(ic + 1) * 512],
                                    start=True,
                                    stop=True,
                                    tile_position=(32 * s, 0),
                                )
                            E = esb.tile([128, 3, 512], FP16, tag="E")
                            nc.scalar.activation(
                                E[:, :nslots, :],
                                L[:, :nslots, :],
                                mybir.ActivationFunctionType.Exp,
                                scale=SCALE,
                            )
                            # previous chunk's epilogue rides in the shadow of
                            # this chunk's first attention group
                            if prev_ic is not None:
                                emit_epilogue(prev_ic)
                                prev_ic = None
                            if pending is not None:
                                if av is None:
                                    av = [ps_av.tile([128, 512], F32, tag=f"av{b}",
                                                     name=f"av{b}_{ic}")
                                          for b in range(2)]
                                emit_av(av, *pending)
                            pending = (E, g, nslots, jb)
                    emit_av(av, *pending)
                    for b in range(2):
                        for c in range(4):
                            nc.vector.tensor_copy(
                                vhat[32 * c:32 * c + 13, b, ic * 512:(ic + 1) * 512],
                                av[b][32 * c:32 * c + 13, :],
                            )
                    prev_ic = ic
                emit_epilogue(3)

    nc.compile()
    return nc


def make_core_inputs(x, W_qkv, W_o):
    """Host-side shard/prepack. Returns list of per-core input dicts."""
    x = np.asarray(x, np.float32)
    W_qkv = np.asarray(W_qkv, np.float32)
    W_o = np.asarray(W_o, np.float32)
    xt = np.ascontiguousarray(x.reshape(S, H).T)  # [H, S]

    sel = np.zeros((128, 128), np.float32)
    for s4 in range(4):
        sel[32 * s4 + 12, 32 * s4:32 * (s4 + 1)] = 1.0

    in_maps = []
    for core in range(N_CORES):
        wqk = np.zeros((H, 2, 3, 128), np.float32)
        wv = np.zeros((H, 96), np.float32)
        wo = np.zeros((2, 128, H), np.float32)
        for g, nslots in GROUPS:
            for s in range(nslots):
                h = _head_of(g, s)
                Hg = HEADS_PER_CORE * core + h
                for t in range(2):
                    wqk[:, t, g, 32 * s:32 * s + 12] = \
                        W_qkv[:, Hg * 36 + t * 12:Hg * 36 + (t + 1) * 12]
        for h in range(HEADS_PER_CORE):
            Hg = HEADS_PER_CORE * core + h
            wv[:, 12 * h:12 * (h + 1)] = W_qkv[:, Hg * 36 + 24:Hg * 36 + 36]
            b, c = divmod(h, 4)
            wo[b, 32 * c:32 * c + 12, :] = W_o[Hg * DK:(Hg + 1) * DK, :]
        in_maps.append({"xt": xt, "wqk": wqk, "wv": wv, "wo": wo, "sel": sel})
    return in_maps


_NC_CACHE = None


def kernel(x, W_qkv, W_o):
    global _NC_CACHE
    if _NC_CACHE is None:
        _NC_CACHE = build_program()
    nc = _NC_CACHE
    in_maps = make_core_inputs(x, W_qkv, W_o)
    res = run_bass_kernel_spmd(nc, in_maps, core_ids=list(range(N_CORES)))
    out = np.zeros((S, H), np.float64)
    for r in res.results:
        out += r["out"].astype(np.float64)
    return out.astype(np.float32).reshape(1, S, H)
